# revision 1
# baseline (speedup 1.0000x reference)
"""Multi-head causal attention (B=2, S=2048, D=1024, H=16, HD=64) on 8 TRN2 cores.

Sharding: core c handles batch b = c//4 and heads 4*(c%4)..4*(c%4)+3.
The reference reshapes [b,h,s,hd] -> [b,s,1024] WITHOUT head transpose-back,
so output rows [128h, 128h+128) of y[b] depend only on head h: each core
produces a disjoint [512, 1024] block of the output. No collectives.

Per-core kernel (all matmuls float32r, N>=256, base partition 0):
  P1 QKV:  Q^T/K^T per head-pair stacked [128, 2048] (head B extracted to a
           base-0 [64, 2048] tile via partition-shifting SBUF->SBUF DMA);
           V packed [128(s), 16 s-tiles x (4 heads x 64 + ones col)].
  P2 attn: per pair, per q-block j (512), per k-tile t (0..4j+3):
           psS2[128,1024] = scores^T both heads; one ACT exp (scale 1/8);
           diagonal tiles masked post-exp by 0/1 mul; PV matmul with
           V|ones (M=65) accumulates attnT + denominator row.
  P3 norm: denom row -> ones-matmul broadcast [64,512] -> reciprocal -> mul.
  P4 proj: y rows = sum_m attnT_norm[:, m::16].T @ Wo[64m:64m+64, :] + bo.
"""

import sys

if "/opt/trn_rl_repo" not in sys.path:
    sys.path.insert(0, "/opt/trn_rl_repo")

from contextlib import ExitStack

import numpy as np

import concourse.bass as bass
import concourse.tile as tile
from concourse.tile import add_dep_helper
from concourse import bacc, mybir

F32 = mybir.dt.float32
F32R = mybir.dt.float32r
EXP = mybir.ActivationFunctionType.Exp

B, S, D, H, HD = 2, 2048, 1024, 16, 64
NC = 8
HPC = 4  # heads per core
CT = D // 128  # 8 contraction tiles
QB = 4  # q-blocks of 512
KT = S // 128  # 16 k-tiles
SCALE = 1.0 / 8.0


def build_nc():
    nc = bacc.Bacc("TRN2", target_bir_lowering=False, debug=False)

    xt = nc.dram_tensor("xt", [128, CT, S], F32R, kind="ExternalInput").ap()
    wq = nc.dram_tensor("wq", [128, 2, CT, 128], F32R, kind="ExternalInput").ap()
    wk = nc.dram_tensor("wk", [128, 2, CT, 128], F32R, kind="ExternalInput").ap()
    wv = nc.dram_tensor("wv", [128, CT, 256], F32R, kind="ExternalInput").ap()
    wo = nc.dram_tensor("wo", [4, 64, 16, 256], F32R, kind="ExternalInput").ap()
    bo = nc.dram_tensor("bo", [D], F32, kind="ExternalInput").ap()
    masks = nc.dram_tensor("masks", [128, 4, 512], F32, kind="ExternalInput").ap()
    ones_v = nc.dram_tensor("ones_v", [128, KT, 4], F32R, kind="ExternalInput").ap()
    oneh = nc.dram_tensor("oneh", [16, 1024], F32R, kind="ExternalInput").ap()
    y = nc.dram_tensor("y", [HPC * 128, D], F32, kind="ExternalOutput").ap()

    with tile.TileContext(nc) as tc, ExitStack() as ctx:
        with ExitStack() as scopeA:
            a_pool = scopeA.enter_context(tc.tile_pool(name="a", bufs=1))

            # V: [128(s_local), 16 s-tiles, 4*65] (col 64 of each group = ones)
            v4 = a_pool.tile([128, KT, 260], F32R, tag="v4")
            nc.sync.dma_start(
                out=v4[:].rearrange("p t (h c) -> p t h c", c=65)[:, :, :, 64:65],
                in_=ones_v.unsqueeze(3),
            )
            qst = [a_pool.tile([128, S], F32R, tag=f"qst{p}", name=f"qst{p}") for p in range(2)]
            kst = [a_pool.tile([128, S], F32R, tag=f"kst{p}", name=f"kst{p}") for p in range(2)]
            qtb = [a_pool.tile([64, S], F32R, tag=f"qtb{p}", name=f"qtb{p}") for p in range(2)]
            ktb = [a_pool.tile([64, S], F32R, tag=f"ktb{p}", name=f"ktb{p}") for p in range(2)]

            # ---- P1: QKV projections
            last_p1 = [None]
            with ExitStack() as scopeB:
                xt_pool = scopeB.enter_context(tc.tile_pool(name="xt", bufs=1))
                w_pool = scopeB.enter_context(tc.tile_pool(name="w", bufs=1))
                ps1 = scopeB.enter_context(tc.tile_pool(name="ps1", bufs=2, space="PSUM"))

                xt_sb = xt_pool.tile([128, CT, S], F32R, tag="xt")
                wv_sb = w_pool.tile([128, CT, 256], F32R, tag="wv")

                def qk_pair(p):
                    wq_sb = w_pool.tile([128, CT, 128], F32R, tag="wq", name=f"wq{p}")
                    nc.sync.dma_start(out=wq_sb[:], in_=wq[:, p])
                    wk_sb = w_pool.tile([128, CT, 128], F32R, tag="wk", name=f"wk{p}")
                    nc.sync.dma_start(out=wk_sb[:], in_=wk[:, p])
                    if p == 0:
                        for quad in range(4):
                            nc.sync.dma_start(
                                out=xt_sb[:, 2 * quad : 2 * quad + 2, :],
                                in_=xt[:, 2 * quad : 2 * quad + 2, :],
                            )
                        nc.sync.dma_start(out=wv_sb[:], in_=wv)
                    # ct-outer with 8 live psum accumulators: PE starts on
                    # the first xt quarter instead of waiting for all of xt
                    psqk = [
                        ps1.tile([128, 512], F32, tag=f"qk{i}", name=f"qk{i}", bufs=1)
                        for i in range(8)
                    ]
                    for ct in range(CT):
                        for i, w_sb in ((0, wq_sb), (4, wk_sb)):
                            for nb in range(QB):
                                nc.tensor.matmul(
                                    psqk[i + nb][:],
                                    w_sb[:, ct, :],
                                    xt_sb[:, ct, bass.ts(nb, 512)],
                                    start=(ct == 0),
                                    stop=(ct == CT - 1),
                                )
                    for i, dst in ((0, qst[p]), (4, kst[p])):
                        for nb in range(QB):
                            nc.vector.tensor_copy(
                                dst[:, bass.ts(nb, 512)], psqk[i + nb][:]
                            )
                    nc.sync.dma_start(out=qtb[p][:], in_=qst[p][64:128, :])
                    ext = nc.sync.dma_start(out=ktb[p][:], in_=kst[p][64:128, :])
                    last_p1[0] = ext

                qk_pair(0)
                for st in range(KT):
                    ps = ps1.tile([128, 256], F32, tag=f"qk{st % 8}", name="psv", bufs=1)
                    for ct in range(CT):
                        nc.tensor.matmul(
                            ps[:],
                            xt_sb[:, ct, bass.ts(st, 128)],
                            wv_sb[:, ct, :],
                            start=(ct == 0),
                            stop=(ct == CT - 1),
                        )
                    nc.vector.tensor_copy(
                        v4[:, st, :].rearrange("p (h c) -> p h c", c=65)[:, :, 0:64],
                        ps[:].rearrange("p (h c) -> p h c", c=64),
                    )
                qk_pair(1)

            # ---- P2/P3/P4 interleaved (pools created after scopeB frees xt/w)
            att = scopeA.enter_context(tc.tile_pool(name="att", bufs=1))
            pt_pool = scopeA.enter_context(tc.tile_pool(name="pt", bufs=3))
            c_pool = scopeA.enter_context(tc.tile_pool(name="c", bufs=2))
            r_pool = scopeA.enter_context(tc.tile_pool(name="r", bufs=2))
            y_pool = scopeA.enter_context(tc.tile_pool(name="y", bufs=2))
            ps2 = scopeA.enter_context(tc.tile_pool(name="ps2", bufs=2, space="PSUM"))
            masks_sb = att.tile([128, 4, 512], F32, tag="masks")
            d1 = nc.sync.dma_start(out=masks_sb[:], in_=masks)
            bo_sb = att.tile([128, D], F32, tag="bo")
            bo_b = bass.AP(tensor=bo.tensor, offset=bo.offset, ap=[[0, 128], [1, D]])
            d2 = nc.sync.dma_start(out=bo_sb[:], in_=bo_b)
            oneh_sb = att.tile([16, 1024], F32R, tag="oneh")
            d3 = nc.sync.dma_start(out=oneh_sb[:], in_=oneh)
            for dd in (d1, d2, d3):
                add_dep_helper(last_p1[0].ins, dd.ins, sync=True, reason="phase order")
            attnT = [att.tile([64, S], F32R, tag=f"attnT{h}", name=f"attnT{h}") for h in range(HPC)]
            dall = [att.tile([8, 512], F32R, tag=f"dall{p}", name=f"dall{p}") for p in range(2)]
            # init: unwritten rows are multiplied by one-hot zeros; must be finite
            for p in range(2):
                nc.sync.dma_start(out=dall[p][:], in_=oneh[0:8, 0:512])

            def proj(h, qq):
                wo_sb = c_pool.tile([64, 16, 256], F32R, tag="wo", name=f"wo{h}{qq}")
                nc.sync.dma_start(out=wo_sb[:], in_=wo[qq])
                psy = ps2.tile([128, 256], F32, tag="psa1", name="psy")
                a = attnT[h][:].rearrange("p (r m) -> p m r", m=16)
                for m in range(16):
                    nc.tensor.matmul(
                        psy[:],
                        a[:, m, :],
                        wo_sb[:, m, :],
                        start=(m == 0),
                        stop=(m == 15),
                    )
                ys = y_pool.tile([128, 256], F32, tag="ys", name="ys")
                nc.vector.tensor_add(ys[:], psy[:], bo_sb[:, bass.ts(qq, 256)])
                nc.sync.dma_start(out=y[bass.ts(h, 128), bass.ts(qq, 256)], in_=ys[:])

            for p in range(2):
                qv = [(qst[p][0:64, :], kst[p][0:64, :]), (qtb[p][:], ktb[p][:])]
                for j in range(QB):
                    psa = [
                        ps2.tile([128, 512], F32, tag=f"psa{q}", name=f"psa{q}", bufs=2)
                        for q in range(2)
                    ]
                    for t in range(4 * j + 4):
                        pss = ps2.tile([128, 1024], F32, tag="pss", name="pss")
                        for q in range(2):
                            qt, kt = qv[q]
                            nc.tensor.matmul(
                                pss[:, bass.ts(q, 512)],
                                kt[:, bass.ts(t, 128)],
                                qt[:, bass.ts(j, 512)],
                                start=True,
                                stop=True,
                            )
                        pt2 = pt_pool.tile([128, 1024], F32R, tag="pt2", name="pt2")
                        nc.scalar.activation(pt2[:], pss[:], EXP, scale=SCALE)
                        r = t - 4 * j
                        if r >= 0:
                            w = 128 * (r + 1)  # cols past w are never masked
                            nc.vector.tensor_mul(
                                pt2[:, 0:w], pt2[:, 0:w], masks_sb[:, r, 0:w]
                            )
                            nc.gpsimd.tensor_mul(
                                pt2[:, 512 : 512 + w],
                                pt2[:, 512 : 512 + w],
                                masks_sb[:, r, 0:w],
                            )
                        for q in range(2):
                            h = 2 * p + q
                            nc.tensor.matmul(
                                psa[q][0:65, :],
                                v4[:, t, bass.ds(65 * h, 65)],
                                pt2[:, bass.ts(q, 512)],
                                start=(t == 0),
                                stop=(t == 4 * j + 3),
                            )
                    for q in range(2):
                        h = 2 * p + q
                        nc.vector.tensor_copy(attnT[h][:, bass.ts(j, 512)], psa[q][0:64, :])
                        dtmp = r_pool.tile([1, 512], F32R, tag="dtmp", name="dtmp")
                        nc.vector.tensor_copy(dtmp[:], psa[q][64:65, :])
                        nc.sync.dma_start(
                            out=dall[p][4 * q + j : 4 * q + j + 1, :], in_=dtmp[:]
                        )
            # normalize all heads (gap-fills under pair-1 attention)
            for h in range(HPC):
                p, q = h // 2, h % 2
                for cb in range(QB):
                    psr = ps2.tile([64, 512], F32, tag="psa0", name="psr")
                    nc.tensor.matmul(
                        psr[:],
                        oneh_sb[0:8, bass.ds(64 * (4 * q + cb), 64)],
                        dall[p][:],
                        start=True, stop=True,
                    )
                    rr = r_pool.tile([64, 512], F32, tag="rr", name="rr")
                    nc.vector.reciprocal(rr[:], psr[:])
                    nc.gpsimd.tensor_mul(
                        attnT[h][:, bass.ts(cb, 512)],
                        attnT[h][:, bass.ts(cb, 512)],
                        rr[:],
                    )
            # output projection: quarter-outer so each wo quarter loads once
            for qq in range(4):
                wo_sb = c_pool.tile([64, 16, 256], F32R, tag="wo", name=f"wo{qq}")
                nc.sync.dma_start(out=wo_sb[:], in_=wo[qq])
                for h in range(HPC):
                    psy = ps2.tile([128, 256], F32, tag="psa1", name="psy")
                    a = attnT[h][:].rearrange("p (r m) -> p m r", m=16)
                    for m in range(16):
                        nc.tensor.matmul(
                            psy[:],
                            a[:, m, :],
                            wo_sb[:, m, :],
                            start=(m == 0),
                            stop=(m == 15),
                        )
                    ys = y_pool.tile([128, 256], F32, tag="ys", name="ys")
                    nc.vector.tensor_add(ys[:], psy[:], bo_sb[:, bass.ts(qq, 256)])
                    nc.sync.dma_start(out=y[bass.ts(h, 128), bass.ts(qq, 256)], in_=ys[:])

    nc.compile()
    return nc


def make_masks():
    kl = np.arange(128)[:, None]
    ql = np.arange(512)[None, :]
    return np.ascontiguousarray(
        np.stack(
            [(128 * r + kl <= ql).astype(np.float32) for r in range(4)]
        ).transpose(1, 0, 2)
    )  # [128, 4, 512]


def prep_core_inputs(c, x, Wq, Wk, Wv, Wo, bo):
    b, g = c // 4, c % 4
    heads = [4 * g + i for i in range(HPC)]
    xt = np.ascontiguousarray(x[b].T.reshape(CT, 128, S).transpose(1, 0, 2))

    def pack_pair(W, p):
        h0, h1 = heads[2 * p], heads[2 * p + 1]
        cols = np.concatenate(
            [W[:, 64 * h0 : 64 * h0 + 64], W[:, 64 * h1 : 64 * h1 + 64]], 1
        )
        return cols.reshape(CT, 128, 128)

    wq = np.ascontiguousarray(
        np.stack([pack_pair(Wq, p) for p in range(2)]).transpose(2, 0, 1, 3)
    )  # [128, 2, CT, 128]
    wk = np.ascontiguousarray(
        np.stack([pack_pair(Wk, p) for p in range(2)]).transpose(2, 0, 1, 3)
    )
    wv = np.ascontiguousarray(
        np.concatenate([Wv[:, 64 * h : 64 * h + 64] for h in heads], 1)
        .reshape(CT, 128, 256)
        .transpose(1, 0, 2)
    )  # [128, CT, 256]
    wo = np.ascontiguousarray(
        Wo.reshape(16, 64, 4, 256).transpose(2, 1, 0, 3)
    )  # [4, 64, 16, 256]
    return {
        "xt": xt,
        "wq": wq,
        "wk": wk,
        "wv": wv,
        "wo": wo,
        "bo": bo,
        "masks": make_masks(),
        "ones_v": np.ones((128, KT, 4), np.float32),
        "oneh": np.kron(np.eye(16, dtype=np.float32), np.ones((1, 64), np.float32)),
    }


_NC_CACHE = []


def kernel(x, Wq, Wk, Wv, Wo, bo):
    from concourse import bass_utils

    x, Wq, Wk, Wv, Wo, bo = (
        np.asarray(x, np.float32),
        np.asarray(Wq, np.float32),
        np.asarray(Wk, np.float32),
        np.asarray(Wv, np.float32),
        np.asarray(Wo, np.float32),
        np.asarray(bo, np.float32),
    )
    if not _NC_CACHE:
        _NC_CACHE.append(build_nc())
    nc = _NC_CACHE[0]
    in_maps = [prep_core_inputs(c, x, Wq, Wk, Wv, Wo, bo) for c in range(NC)]
    res = bass_utils.run_bass_kernel_spmd(nc, in_maps, core_ids=list(range(NC)))
    out = np.empty((B, S, D), np.float32)
    for c in range(NC):
        b, g = c // 4, c % 4
        out[b, 512 * g : 512 * (g + 1), :] = res.results[c]["y"]
    return out



# revision 3
# speedup vs baseline: 1.0769x; 1.0769x over previous
"""Multi-head causal attention (B=2, S=2048, D=1024, H=16, HD=64) on 8 TRN2 cores.

Sharding: core c handles batch b = c//4 and heads 4*(c%4)..4*(c%4)+3.
The reference reshapes [b,h,s,hd] -> [b,s,1024] WITHOUT head transpose-back,
so output rows [128h, 128h+128) of y[b] depend only on head h: each core
produces a disjoint [512, 1024] block of the output. No collectives.

v2 (bf16 + trim + interleave):
  - All weight/activation DRAM inputs in bf16 (host-converted); matmuls run
    at 1 cyc/row in the cost model even for N<256 (fp32r needs N>=256).
  - Diagonal score tiles trimmed: scores matmul, exp, and PV only cover
    q-cols >= 128r of the 512-q block; the partial 128x128 triangle block is
    masked post-exp by one strided bf16 DVE multiply (both heads at once).
  - Phase interleave for PE occupancy: pair-1 Q/K projections are emitted
    inside pair-0's attention j-loop (they fill the Act-bound bubbles);
    pair-0 normalize + output projection are emitted inside pair-1's
    attention j-loop.
  - Output projection at K=128: attnT2b[h] is [128, 2048] with partitions
    64:128 holding a 1-col-left-shifted copy of rows 0:64 (SBUF->SBUF DMA),
    so lhsT [128,128] packs head-chunk pairs (m, m+1) and Wo contracts in 8
    chunks of 128 instead of 16 of 64 (halves proj PE rows).
  - Softmax denominators ride the PV matmul as a 65th V column; rows are
    gathered (SBUF->SBUF shift DMA) into dall[8,512] per pair, one batched
    reciprocal per pair, then broadcast back via one-hot matmul and applied
    by DVE/Pool multiplies.
"""

import sys

if "/opt/trn_rl_repo" not in sys.path:
    sys.path.insert(0, "/opt/trn_rl_repo")

from contextlib import ExitStack

import numpy as np
import ml_dtypes

import concourse.bass as bass
import concourse.tile as tile
from concourse import bacc, mybir

F32 = mybir.dt.float32
F32R = mybir.dt.float32r
BF16 = mybir.dt.bfloat16
EXP = mybir.ActivationFunctionType.Exp

B, S, D, H, HD = 2, 2048, 1024, 16, 64
NC = 8
HPC = 4  # heads per core
CT = D // 128  # 8 contraction tiles
QB = 4  # q-blocks of 512
KT = S // 128  # 16 k-tiles
SCALE = 1.0 / 8.0
NPBF16 = ml_dtypes.bfloat16


def build_nc():
    nc = bacc.Bacc("TRN2", target_bir_lowering=False, debug=False)

    xt = nc.dram_tensor("xt", [128, CT, S], BF16, kind="ExternalInput").ap()
    wq = nc.dram_tensor("wq", [128, 2, CT, 128], BF16, kind="ExternalInput").ap()
    wk = nc.dram_tensor("wk", [128, 2, CT, 128], BF16, kind="ExternalInput").ap()
    wv = nc.dram_tensor("wv", [128, CT, 256], BF16, kind="ExternalInput").ap()
    wo2 = nc.dram_tensor("wo2", [128, 8, 4, 256], BF16, kind="ExternalInput").ap()
    bo = nc.dram_tensor("bo", [D], F32, kind="ExternalInput").ap()
    masks = nc.dram_tensor("masks", [128, 2, 128], BF16, kind="ExternalInput").ap()
    oneh = nc.dram_tensor("oneh", [8, 512], BF16, kind="ExternalInput").ap()
    y = nc.dram_tensor("y", [HPC * 128, D], F32, kind="ExternalOutput").ap()

    with tile.TileContext(nc) as tc, ExitStack() as ctx:
        a_pool = ctx.enter_context(tc.tile_pool(name="a", bufs=1))

        # ---- resident SBUF tensors
        xt_sb = a_pool.tile([128, CT, S], BF16, tag="xt")
        wq_sb = a_pool.tile([128, 2, CT, 128], BF16, tag="wq")
        wk_sb = a_pool.tile([128, 2, CT, 128], BF16, tag="wk")
        wv_sb = a_pool.tile([128, CT, 256], BF16, tag="wv")
        wo2_sb = a_pool.tile([128, 8, 4, 256], BF16, tag="wo2")
        masks_sb = a_pool.tile([128, 2, 128], BF16, tag="masks")
        oneh_sb = a_pool.tile([8, 512], BF16, tag="oneh")
        bo_sb = a_pool.tile([128, D], F32, tag="bo")
        # V packed [128(s_local), 16 s-tiles, 4*(64+ones col)] bf16
        v4 = a_pool.tile([128, KT, 260], BF16, tag="v4")
        qst = [a_pool.tile([128, S], BF16, tag=f"qst{p}", name=f"qst{p}") for p in range(2)]
        kst = [a_pool.tile([128, S], BF16, tag=f"kst{p}", name=f"kst{p}") for p in range(2)]
        qtb = [a_pool.tile([64, S], BF16, tag=f"qtb{p}", name=f"qtb{p}") for p in range(2)]
        ktb = [a_pool.tile([64, S], BF16, tag=f"ktb{p}", name=f"ktb{p}") for p in range(2)]
        # attnT2b[h]: rows 0:64 = attn^T (hd x q), rows 64:128 = 1-col-left-
        # shifted copy (for K=128 proj lhsT)
        attnT2b = [
            a_pool.tile([128, S], BF16, tag=f"at{h}", name=f"at{h}") for h in range(HPC)
        ]
        dall = [a_pool.tile([8, 512], BF16, tag=f"dall{p}", name=f"dall{p}") for p in range(2)]
        dallr = [
            a_pool.tile([8, 512], F32R, tag=f"dallr{p}", name=f"dallr{p}") for p in range(2)
        ]

        # ---- input DMAs (SP queue; order = need order)
        nc.sync.dma_start(out=masks_sb[:], in_=masks)
        for quad in range(4):
            nc.sync.dma_start(
                out=xt_sb[:, 2 * quad : 2 * quad + 2, :],
                in_=xt[:, 2 * quad : 2 * quad + 2, :],
            )
        nc.sync.dma_start(out=wq_sb[:], in_=wq)
        nc.sync.dma_start(out=wk_sb[:], in_=wk)
        nc.sync.dma_start(out=wv_sb[:], in_=wv)
        nc.sync.dma_start(out=oneh_sb[:], in_=oneh)
        bo_b = bass.AP(tensor=bo.tensor, offset=bo.offset, ap=[[0, 128], [1, D]])
        nc.sync.dma_start(out=bo_sb[:], in_=bo_b)
        nc.sync.dma_start(out=wo2_sb[:], in_=wo2)
        # ones column of v4 via memset (strided view)
        nc.gpsimd.memset(
            v4[:].rearrange("p t (h c) -> p t h c", c=65)[:, :, :, 64:65], 1.0
        )

        y_pool = ctx.enter_context(tc.tile_pool(name="y", bufs=2))
        pt_pool = ctx.enter_context(tc.tile_pool(name="pt", bufs=3))

        # ---- P1 pair 0: Q/K ct-outer with 8 live psum accumulators, then V
        with ExitStack() as scope1:
            ps1 = scope1.enter_context(tc.tile_pool(name="ps1", bufs=2, space="PSUM"))
            psqk = [
                ps1.tile([128, 512], F32, tag=f"qk{i}", name=f"qk{i}", bufs=1)
                for i in range(8)
            ]
            for ct in range(CT):
                for i, w_sb in ((0, wq_sb), (4, wk_sb)):
                    for nb in range(QB):
                        nc.tensor.matmul(
                            psqk[i + nb][:],
                            w_sb[:, 0, ct, :],
                            xt_sb[:, ct, bass.ts(nb, 512)],
                            start=(ct == 0),
                            stop=(ct == CT - 1),
                        )
            for i, dst in ((0, qst[0]), (4, kst[0])):
                for nb in range(QB):
                    nc.vector.tensor_copy(dst[:, bass.ts(nb, 512)], psqk[i + nb][:])
            nc.sync.dma_start(out=qtb[0][:], in_=qst[0][64:128, :])
            nc.sync.dma_start(out=ktb[0][:], in_=kst[0][64:128, :])
            # V for all 4 heads (st-outer, ct accumulation)
            for st in range(KT):
                ps = ps1.tile([128, 256], F32, tag=f"qk{st % 8}", name="psv", bufs=1)
                for ct in range(CT):
                    nc.tensor.matmul(
                        ps[:],
                        xt_sb[:, ct, bass.ts(st, 128)],
                        wv_sb[:, ct, :],
                        start=(ct == 0),
                        stop=(ct == CT - 1),
                    )
                nc.gpsimd.tensor_copy(
                    v4[:, st, :].rearrange("p (h c) -> p h c", c=65)[:, :, 0:64],
                    ps[:].rearrange("p (h c) -> p h c", c=64),
                )

        # ---- P2: attention, with interleaved fillers
        ps2 = ctx.enter_context(tc.tile_pool(name="ps2", bufs=2, space="PSUM"))

        def attn_tile(p, j, t, qv):
            """One (k-tile, q-block) step for head pair p: scores^T both
            heads -> exp -> mask (diagonal) -> PV accumulate."""
            r = t - 4 * j
            c0 = 128 * r if r > 0 else 0
            pss = ps2.tile([128, 1024], F32, tag="pss", name="pss")
            for q in range(2):
                qt, kt = qv[q]
                nc.tensor.matmul(
                    pss[:, 512 * q + c0 : 512 * (q + 1)],
                    kt[:, bass.ts(t, 128)],
                    qt[:, 512 * j + c0 : 512 * (j + 1)],
                    start=True,
                    stop=True,
                )
            pt2 = pt_pool.tile([128, 1024], BF16, tag="pt2", name="pt2")
            if r < 0:
                nc.scalar.activation(pt2[:], pss[:], EXP, scale=SCALE)
            else:
                pv = pss[:].rearrange("p (h c) -> p h c", c=512)[:, :, c0:512]
                ov = pt2[:].rearrange("p (h c) -> p h c", c=512)[:, :, c0:512]
                nc.scalar.activation(ov, pv, EXP, scale=SCALE)
                mv = pt2[:].rearrange("p (h c) -> p h c", c=512)[:, :, c0 : c0 + 128]
                nc.vector.tensor_mul(mv, mv, masks_sb[:])
            return pss, pt2

        def attn_pv(p, j, t, pt2, psa):
            r = t - 4 * j
            c0 = 128 * r if r > 0 else 0
            last = t == 4 * j + 3
            for q in range(2):
                h = 2 * p + q
                vsl = v4[:, t, bass.ds(65 * h, 65)]
                if r >= 0:
                    # split stop: [c0, c0+128) is final here; rest continues
                    nc.tensor.matmul(
                        psa[q][0:65, c0 : c0 + 128],
                        vsl,
                        pt2[:, 512 * q + c0 : 512 * q + c0 + 128],
                        start=(t == 0),
                        stop=True,
                    )
                    if not last:
                        nc.tensor.matmul(
                            psa[q][0:65, c0 + 128 : 512],
                            vsl,
                            pt2[:, 512 * q + c0 + 128 : 512 * (q + 1)],
                            start=(t == 0),
                            stop=False,
                        )
                else:
                    nc.tensor.matmul(
                        psa[q][0:65, :],
                        vsl,
                        pt2[:, bass.ts(q, 512)],
                        start=(t == 0),
                        stop=False,
                    )

        def attn_j(p, j, qv, fillers):
            psa = [
                ps2.tile([128, 512], F32, tag=f"psa{q}", name=f"psa{q}", bufs=2)
                for q in range(2)
            ]
            for t in range(4 * j + 4):
                pss, pt2 = attn_tile(p, j, t, qv)
                attn_pv(p, j, t, pt2, psa)
            for q in range(2):
                h = 2 * p + q
                nc.vector.tensor_copy(
                    attnT2b[h][0:65, bass.ts(j, 512)], psa[q][0:65, :]
                )
                nc.sync.dma_start(
                    out=dall[p][4 * q + j : 4 * q + j + 1, :],
                    in_=attnT2b[h][64:65, bass.ts(j, 512)],
                )
            for f in fillers:
                f()

        def qk1_chunk(nb):
            """Pair-1 Q/K projection for one 512-col s-block (rides pss tag)."""
            pssqk = ps2.tile([128, 1024], F32, tag="pss", name="pssqk")
            for half, w_sb in ((0, wq_sb), (1, wk_sb)):
                for ct in range(CT):
                    nc.tensor.matmul(
                        pssqk[:, bass.ts(half, 512)],
                        w_sb[:, 1, ct, :],
                        xt_sb[:, ct, bass.ts(nb, 512)],
                        start=(ct == 0),
                        stop=(ct == CT - 1),
                    )
            nc.vector.tensor_copy(qst[1][:, bass.ts(nb, 512)], pssqk[:, 0:512])
            nc.vector.tensor_copy(kst[1][:, bass.ts(nb, 512)], pssqk[:, 512:1024])
            nc.sync.dma_start(
                out=qtb[1][:, bass.ts(nb, 512)], in_=qst[1][64:128, bass.ts(nb, 512)]
            )
            nc.sync.dma_start(
                out=ktb[1][:, bass.ts(nb, 512)], in_=kst[1][64:128, bass.ts(nb, 512)]
            )

        def recip(p):
            with nc.allow_low_precision(reason="softmax denom reciprocal in f32r"):
                nc.vector.reciprocal(dallr[p][:], dall[p][:])

        def norm_h(h, mul_engine):
            """Broadcast 1/denom rows to [64, 512] blocks and scale attnT."""
            p, q = h // 2, h % 2
            for cb2 in range(2):  # two cb per psum tile
                psr = ps2.tile([128, 1024], F32, tag="pss", name="psr")
                for k in range(2):
                    cb = 2 * cb2 + k
                    nc.tensor.matmul(
                        psr[0:64, bass.ts(k, 512)],
                        oneh_sb[0:8, bass.ds(64 * (4 * q + cb), 64)],
                        dallr[p][:],
                        start=True,
                        stop=True,
                    )
                for k in range(2):
                    cb = 2 * cb2 + k
                    mul_engine.tensor_mul(
                        attnT2b[h][0:64, bass.ts(cb, 512)],
                        attnT2b[h][0:64, bass.ts(cb, 512)],
                        psr[0:64, bass.ts(k, 512)],
                    )
            for cb in range(4):
                # shifted duplicate for K=128 proj: row 64+d col c = row d col c+1
                nc.sync.dma_start(
                    out=attnT2b[h][64:128, 512 * cb : 512 * cb + 511],
                    in_=attnT2b[h][0:64, 512 * cb + 1 : 512 * (cb + 1)],
                )

        def proj_qq(h, qq):
            a2 = attnT2b[h][:].rearrange("p (r s) -> p s r", s=16)
            psy = ps2.tile([128, 1024], F32, tag="pss", name="psy")
            for mp in range(8):
                nc.tensor.matmul(
                    psy[:, 0:256],
                    a2[:, 2 * mp, :],
                    wo2_sb[:, mp, qq, :],
                    start=(mp == 0),
                    stop=(mp == 7),
                )
            ys = y_pool.tile([128, 256], F32, tag="ys", name="ys")
            nc.vector.tensor_add(ys[:], psy[:, 0:256], bo_sb[:, bass.ts(qq, 256)])
            nc.sync.dma_start(out=y[bass.ts(h, 128), bass.ts(qq, 256)], in_=ys[:])

        qv0 = [(qst[0][0:64, :], kst[0][0:64, :]), (qtb[0][:], ktb[0][:])]
        qv1 = [(qst[1][0:64, :], kst[1][0:64, :]), (qtb[1][:], ktb[1][:])]

        # pair-0 attention; pair-1 QK fills the Act-bound PE bubbles
        for j in range(QB):
            attn_j(0, j, qv0, [lambda nb=j: qk1_chunk(nb)])

        # pair-1 attention; pair-0 norm + proj fill the bubbles
        fill1 = [
            [lambda: recip(0), lambda: norm_h(0, nc.gpsimd), lambda: proj_qq(0, 0)],
            [lambda: norm_h(1, nc.gpsimd), lambda: proj_qq(0, 1), lambda: proj_qq(0, 2)],
            [lambda: proj_qq(0, 3), lambda: proj_qq(1, 0), lambda: proj_qq(1, 1)],
            [lambda: proj_qq(1, 2), lambda: proj_qq(1, 3)],
        ]
        for j in range(QB):
            attn_j(1, j, qv1, fill1[j])

        # tail: pair-1 norm + proj
        recip(1)
        norm_h(2, nc.vector)
        norm_h(3, nc.gpsimd)
        for qq in range(4):
            proj_qq(2, qq)
        for qq in range(4):
            proj_qq(3, qq)

    nc.compile()
    return nc


def make_masks():
    kl = np.arange(128)[:, None]
    cl = np.arange(128)[None, :]
    tri = (kl <= cl).astype(NPBF16)  # [128 k, 128 c]
    return np.ascontiguousarray(np.stack([tri, tri], 1))  # [128, 2, 128]


def prep_core_inputs(c, x, Wq, Wk, Wv, Wo, bo):
    b, g = c // 4, c % 4
    heads = [4 * g + i for i in range(HPC)]
    xt = np.ascontiguousarray(
        x[b].T.reshape(CT, 128, S).transpose(1, 0, 2).astype(NPBF16)
    )

    def pack_pair(W, p):
        h0, h1 = heads[2 * p], heads[2 * p + 1]
        cols = np.concatenate(
            [W[:, 64 * h0 : 64 * h0 + 64], W[:, 64 * h1 : 64 * h1 + 64]], 1
        )
        return cols.reshape(CT, 128, 128)

    wq = np.ascontiguousarray(
        np.stack([pack_pair(Wq, p) for p in range(2)]).transpose(2, 0, 1, 3)
    ).astype(NPBF16)  # [128, 2, CT, 128]
    wk = np.ascontiguousarray(
        np.stack([pack_pair(Wk, p) for p in range(2)]).transpose(2, 0, 1, 3)
    ).astype(NPBF16)
    wv = np.ascontiguousarray(
        np.concatenate([Wv[:, 64 * h : 64 * h + 64] for h in heads], 1)
        .reshape(CT, 128, 256)
        .transpose(1, 0, 2)
    ).astype(NPBF16)  # [128, CT, 256]
    # wo2[d, mp, qq, :] = Wo[128*mp + d, 256*qq : 256*(qq+1)]
    wo2 = np.ascontiguousarray(
        Wo.reshape(8, 128, 4, 256).transpose(1, 0, 2, 3)
    ).astype(NPBF16)  # [128, 8, 4, 256]
    oneh = np.kron(np.eye(8, dtype=np.float32), np.ones((1, 64), np.float32)).astype(
        NPBF16
    )  # [8, 512]
    return {
        "xt": xt,
        "wq": wq,
        "wk": wk,
        "wv": wv,
        "wo2": wo2,
        "bo": bo,
        "masks": make_masks(),
        "oneh": oneh,
    }


_NC_CACHE = []


def kernel(x, Wq, Wk, Wv, Wo, bo):
    from concourse import bass_utils

    x, Wq, Wk, Wv, Wo, bo = (
        np.asarray(x, np.float32),
        np.asarray(Wq, np.float32),
        np.asarray(Wk, np.float32),
        np.asarray(Wv, np.float32),
        np.asarray(Wo, np.float32),
        np.asarray(bo, np.float32),
    )
    if not _NC_CACHE:
        _NC_CACHE.append(build_nc())
    nc = _NC_CACHE[0]
    in_maps = [prep_core_inputs(c, x, Wq, Wk, Wv, Wo, bo) for c in range(NC)]
    res = bass_utils.run_bass_kernel_spmd(nc, in_maps, core_ids=list(range(NC)))
    out = np.empty((B, S, D), np.float32)
    for c in range(NC):
        b, g = c // 4, c % 4
        out[b, 512 * g : 512 * (g + 1), :] = res.results[c]["y"]
    return out
